# revision 1
# baseline (speedup 1.0000x reference)
"""YOLO-v2 loss kernel for Trainium2 (8 NeuronCores, data-parallel over batch).

Layout insight: pyolos [B, 425, 26, 26] is [B, ch*5anc, hw] with plane = c*5+a.
The loss needs:
  - conf channel (planes 0..4) densely: sum of sigmoid(conf)^2 over all
    positions (background term; gconf == 0 exactly wherever no GT matched).
  - cls/txywh channels only at the <=8 matched (cell, anchor) slots per image.
So each core reads 16 conf-plane blocks (216KB) + an indirect gather of
128 slots x 85 channels instead of the full 18.4MB chunk.

Per-core partial sums (8 f32) are combined on the host (the all-reduce-mean
step of the data-parallel recipe).
"""

import numpy as np

from concourse import bass, mybir
from concourse.bass_utils import run_bass_kernel_spmd
from concourse.masks import make_identity
from concourse.tile import TileContext

F32 = mybir.dt.float32
I32 = mybir.dt.int32
AF = mybir.ActivationFunctionType
OP = mybir.AluOpType
AX = mybir.AxisListType

NC = 8                 # cores
B = 128                # batch
BL = B // NC           # images per core (16)
NGT = 8                # GTs per image
S = BL * NGT           # slots per core (128)
GRID = 26
HW = GRID * GRID       # 676
NCH = 85               # conf + 80 cls + 4 txywh
NANC = 5
IMG = 425 * HW         # elements per image (287300)
EPS = 1e-7
ANC = np.array([[0.05, 0.07], [0.12, 0.15], [0.25, 0.30],
                [0.45, 0.50], [0.80, 0.85]], np.float32)

# ---- consts tensor column layout ----
C_UT = 0            # [128] strict-upper-triangular within image blocks
C_IOTA5 = 128       # [5]
C_AW = 133          # [5]
C_AH = 138          # [5]
C_AREA = 143        # [5]
C_IOTA80 = 148      # [80]
C_POW2 = 228        # [5]
C_POW2I = 233       # [5]
C_CHOFF = 238       # [85] per-slot channel offsets (incl. image base)
C_ONES = 323        # [1]
C_IOTA26 = 324      # [26]
C_HALF = 350        # [1] = 0.5
C_GB = 351          # [4] per-slot gbox ltrb
C_LBL = 355         # [1] per-slot label
C_IOTAM99 = 356     # [5] iota5 - 99
NCONST = 361
# gathered channel order: [conf, tx, ty, tw, th, cls0..cls79]
CH_ORDER = [0, 81, 82, 83, 84] + list(range(1, 81))


def _make_consts(gbx_core: np.ndarray, lbl_core: np.ndarray) -> np.ndarray:
    ct = np.zeros((S, NCONST), np.float32)
    ct[:, C_GB:C_GB + 4] = gbx_core
    ct[:, C_LBL] = lbl_core
    i = np.arange(S)
    j = np.arange(S)
    ct[:, C_UT:C_UT + S] = ((i[:, None] // NGT == j[None, :] // NGT)
                            & (j[None, :] > i[:, None])).astype(np.float32)
    ct[:, C_IOTA5:C_IOTA5 + 5] = np.arange(5, dtype=np.float32)[None, :]
    ct[:, C_AW:C_AW + 5] = ANC[:, 0][None, :]
    ct[:, C_AH:C_AH + 5] = ANC[:, 1][None, :]
    ct[:, C_AREA:C_AREA + 5] = (ANC[:, 0] * ANC[:, 1])[None, :]
    ct[:, C_IOTA80:C_IOTA80 + 80] = np.arange(80, dtype=np.float32)[None, :]
    ct[:, C_POW2:C_POW2 + 5] = (2.0 ** np.arange(5))[None, :]
    ct[:, C_POW2I:C_POW2I + 5] = (0.5 ** np.arange(5))[None, :]
    ct[:, C_CHOFF:C_CHOFF + NCH] = ((i // NGT)[:, None] * IMG
                                    + np.array(CH_ORDER)[None, :] * (5 * HW))
    ct[:, C_ONES] = 1.0
    ct[:, C_IOTA26:C_IOTA26 + GRID] = np.arange(GRID, dtype=np.float32)[None]
    ct[:, C_HALF] = 0.5
    ct[:, C_IOTAM99:C_IOTAM99 + 5] = np.arange(5, dtype=np.float32)[None] - 99.0
    return ct


def _split_multiwaits(nc: bass.Bass, k: int = 1) -> None:
    """This walrus build rejects instructions with >~2 sync waits; hoist
    extra waits onto preceding same-engine NoOps (equivalent for monotone
    sem-ge waits)."""
    for fn in nc.m.functions:
        for bb in fn.blocks:
            out = []
            for inst in bb.instructions:
                si = inst.sync_info
                waits = list(si.on_wait) if si is not None and si.on_wait else []
                if len(waits) > k:
                    for i, w in enumerate(waits[:-k]):
                        out.append(mybir.InstNoOp(
                            name=f"{inst.name}-wsplit{i}",
                            engine=inst.engine,
                            bass_nofuse=True,
                            sync_info=mybir.SyncInfo(on_wait=[w],
                                                     on_update=[]),
                        ))
                    inst.sync_info = mybir.SyncInfo(
                        on_wait=waits[-k:], on_update=list(si.on_update))
                out.append(inst)
            bb.instructions = out


def build_bass() -> bass.Bass:
    nc = bass.Bass()
    py = nc.declare_dram_parameter("pyolos", [BL, 425, HW], F32, isOutput=False)
    cn = nc.declare_dram_parameter("consts", [S, NCONST], F32, isOutput=False)
    out = nc.declare_dram_parameter("out", [1, 8], F32, isOutput=True)
    py_flat = py[:, :, :].rearrange("a b c -> (a b c)")

    with TileContext(nc) as tc:
        with (
            tc.tile_pool(name="sb", bufs=1) as sb,
            tc.tile_pool(name="ps", bufs=1, space="PSUM") as ps,
        ):
            ct = sb.tile([S, NCONST], F32, name="ct")
            nc.sync.dma_start(out=ct[:], in_=cn[:, :])
            ident = sb.tile([S, S], F32, name="ident")
            make_identity(nc, ident[:])

            # ---------------- dense conf term ----------------
            conf = sb.tile([BL * 5, HW], F32, name="conf")
            nc.sync.dma_start(out=conf[:], in_=py[:, 0:5, :])
            # sigmoid(x)^2 = exp(-2*softplus(-x)); Exp/Ln only so every ACT
            # op in the kernel shares one table set (no per-op table reloads)
            sigc = sb.tile([BL * 5, HW], F32, name="sigc")
            nc.scalar.activation(sigc[:], conf[:], AF.Exp, scale=-1.0)
            nc.scalar.activation(sigc[:], sigc[:], AF.Ln, bias=1.0)
            sq80 = sb.tile([BL * 5, HW], F32, name="sq80")
            densesq = sb.tile([BL * 5, 1], F32, name="densesq")
            nc.scalar.activation(sq80[:], sigc[:], AF.Exp, scale=-2.0,
                                 accum_out=densesq[:])

            # ---------------- matching (slot layout [128, *]) ----------------
            def tt(shape, tag):
                return sb.tile(shape, F32, name=tag)

            c26 = tt([S, 2], "c26")
            nc.vector.tensor_tensor(out=c26[:], in0=ct[:, C_GB:C_GB + 2],
                                    in1=ct[:, C_GB + 2:C_GB + 4], op=OP.add)
            nc.vector.tensor_scalar_mul(c26[:], c26[:], 13.0)
            wh = tt([S, 2], "wh")
            nc.vector.tensor_tensor(out=wh[:], in0=ct[:, C_GB + 2:C_GB + 4],
                                    in1=ct[:, C_GB:C_GB + 2], op=OP.subtract)

            # floor(c26) via compare-count: sum_k [iota26 <= x] - 1
            colrow = tt([S, 2], "colrow")
            ge26 = tt([S, GRID], "ge26")
            for d in range(2):
                nc.vector.tensor_scalar(ge26[:], ct[:, C_IOTA26:C_IOTA26 + GRID],
                                        c26[:, d:d + 1], None, OP.is_le)
                nc.vector.tensor_reduce(colrow[:, d:d + 1], ge26[:], AX.X,
                                        OP.add)
            nc.vector.tensor_scalar(colrow[:], colrow[:], -1.0, None, OP.add)
            txy = tt([S, 2], "txy")
            nc.vector.tensor_tensor(out=txy[:], in0=c26[:], in1=colrow[:],
                                    op=OP.subtract)
            cell = tt([S, 1], "cell")
            nc.vector.tensor_scalar_mul(cell[:], colrow[:, 1:2], float(GRID))
            nc.vector.tensor_tensor(out=cell[:], in0=cell[:],
                                    in1=colrow[:, 0:1], op=OP.add)

            inw = tt([S, 5], "inw")
            nc.vector.tensor_scalar(inw[:], ct[:, C_AW:C_AW + 5],
                                    wh[:, 0:1], None, OP.min)
            inh = tt([S, 5], "inh")
            nc.vector.tensor_scalar(inh[:], ct[:, C_AH:C_AH + 5],
                                    wh[:, 1:2], None, OP.min)
            inter = tt([S, 5], "inter")
            nc.vector.tensor_tensor(out=inter[:], in0=inw[:], in1=inh[:],
                                    op=OP.mult)
            areag = tt([S, 1], "areag")
            nc.vector.tensor_tensor(out=areag[:], in0=wh[:, 0:1],
                                    in1=wh[:, 1:2], op=OP.mult)
            den = tt([S, 5], "den")
            nc.vector.tensor_tensor(out=den[:], in0=ct[:, C_AREA:C_AREA + 5],
                                    in1=inter[:], op=OP.subtract)
            nc.vector.tensor_scalar(den[:], den[:], areag[:, 0:1], EPS,
                                    OP.add, OP.add)
            deni = tt([S, 5], "deni")
            nc.vector.reciprocal(deni[:], den[:])
            iou2 = tt([S, 5], "iou2")
            nc.vector.tensor_tensor(out=iou2[:], in0=inter[:], in1=deni[:],
                                    op=OP.mult)
            mign = tt([S, 5], "mign")
            nc.vector.tensor_scalar(mign[:], iou2[:], 0.5, None, OP.is_gt)
            mx = tt([S, 1], "mx")
            nc.vector.tensor_reduce(mx[:], iou2[:], AX.X, OP.max)
            eqm = tt([S, 5], "eqm")
            nc.vector.tensor_scalar(eqm[:], iou2[:], mx[:, 0:1], None,
                                    OP.is_equal)
            # first argmax: min over (iota if eq else 99)
            tsel = tt([S, 5], "tsel")
            nc.vector.tensor_tensor(out=tsel[:], in0=ct[:, C_IOTAM99:C_IOTAM99 + 5],
                                    in1=eqm[:], op=OP.mult)
            nc.vector.tensor_scalar(tsel[:], tsel[:], 99.0, None, OP.add)
            idxm = tt([S, 1], "idxm")
            nc.vector.tensor_reduce(idxm[:], tsel[:], AX.X, OP.min)
            acell = tt([S, 1], "acell")
            nc.vector.tensor_scalar_mul(acell[:], idxm[:], float(HW))
            nc.vector.tensor_tensor(out=acell[:], in0=acell[:], in1=cell[:],
                                    op=OP.add)
            offs_f = tt([S, NCH], "offs_f")
            nc.vector.tensor_scalar(offs_f[:], ct[:, C_CHOFF:C_CHOFF + NCH],
                                    acell[:, 0:1], None, OP.add)
            offs_i = sb.tile([S, NCH], I32, name="offs_i")
            nc.vector.tensor_copy(offs_i[:], offs_f[:])
            pf = sb.tile([S, NCH], F32, name="pf")
            nc.gpsimd.indirect_dma_start(
                out=pf[:], out_offset=None,
                in_=py_flat.rearrange("(a b) -> a b", b=1),
                in_offset=bass.IndirectOffsetOnAxis(ap=offs_i[:, :],
                                                    axis=0))

            # ---------------- per-slot loss terms ----------------
            # gathered order: [conf, tx, ty, tw, th, cls0..79]
            onehot5 = tt([S, 5], "onehot5")
            nc.vector.tensor_scalar(onehot5[:], ct[:, C_IOTA5:C_IOTA5 + 5],
                                    idxm[:, 0:1], None, OP.is_equal)

            ancsel = tt([S, 2], "ancsel")
            scr5 = tt([S, 5], "scr5")
            nc.vector.tensor_tensor(out=scr5[:], in0=onehot5[:],
                                    in1=ct[:, C_AW:C_AW + 5], op=OP.mult)
            nc.vector.tensor_reduce(ancsel[:, 0:1], scr5[:], AX.X, OP.add)
            nc.vector.tensor_tensor(out=scr5[:], in0=onehot5[:],
                                    in1=ct[:, C_AH:C_AH + 5], op=OP.mult)
            nc.vector.tensor_reduce(ancsel[:, 1:2], scr5[:], AX.X, OP.add)
            ancinv = tt([S, 2], "ancinv")
            nc.vector.reciprocal(ancinv[:], ancsel[:])
            twh = tt([S, 2], "twh")
            nc.vector.tensor_tensor(out=twh[:], in0=wh[:], in1=ancinv[:],
                                    op=OP.mult)
            nc.scalar.activation(twh[:], twh[:], AF.Ln)
            weight = tt([S, 1], "weight")
            nc.vector.tensor_scalar(weight[:], areag[:], -1.0, 2.0,
                                    OP.mult, OP.add)

            key = tt([S, 1], "key")
            nc.vector.tensor_scalar_mul(key[:], cell[:], 5.0)
            nc.vector.tensor_tensor(out=key[:], in0=key[:], in1=idxm[:],
                                    op=OP.add)

            # ---------------- cross-slot logic (PE transposes) -------------
            def transpose_col(src, tag):
                p = ps.tile([S, S], F32, name=tag + "_p")
                nc.tensor.transpose(out=p[:],
                                    in_=src[:, 0:1].to_broadcast([S, S]),
                                    identity=ident[:])
                t = sb.tile([S, S], F32, name=tag)
                nc.vector.tensor_copy(t[:], p[:])
                return t

            keyT = transpose_col(key, "keyT")
            cellT = transpose_col(cell, "cellT")

            eqkey = tt([S, S], "eqkey")
            nc.vector.tensor_scalar(eqkey[:], keyT[:], key[:, 0:1], None,
                                    OP.is_equal)
            nc.vector.tensor_tensor(out=eqkey[:], in0=eqkey[:],
                                    in1=ct[:, C_UT:C_UT + S], op=OP.mult)
            ovw = tt([S, 1], "ovw")
            nc.vector.tensor_reduce(ovw[:], eqkey[:], AX.X, OP.max)
            lastw = tt([S, 1], "lastw")
            nc.vector.tensor_scalar(lastw[:], ovw[:], -1.0, 1.0,
                                    OP.mult, OP.add)

            # bit[i, j] = mign[j, anc_i] via PE: onehot5^T (x) mign^T matmul
            oh5T_p = ps.tile([5, S], F32, name="oh5T_p")
            nc.tensor.transpose(out=oh5T_p[:], in_=onehot5[:],
                                identity=ident[:])
            oh5T = sb.tile([5, S], F32, name="oh5T")
            nc.vector.tensor_copy(oh5T[:], oh5T_p[:])
            mignT_p = ps.tile([5, S], F32, name="mignT_p")
            nc.tensor.transpose(out=mignT_p[:], in_=mign[:],
                                identity=ident[:])
            mignT = sb.tile([5, S], F32, name="mignT")
            nc.vector.tensor_copy(mignT[:], mignT_p[:])
            bit_p = ps.tile([S, S], F32, name="bit_p")
            nc.tensor.matmul(out=bit_p[:], lhsT=oh5T[:], rhs=mignT[:],
                             start=True, stop=True)
            bit = tt([S, S], "bit")
            nc.vector.tensor_copy(bit[:], bit_p[:])

            eqc = tt([S, S], "eqc")
            nc.vector.tensor_scalar(eqc[:], cellT[:], cell[:, 0:1], None,
                                    OP.is_equal)
            nc.vector.tensor_tensor(out=eqc[:], in0=eqc[:], in1=bit[:],
                                    op=OP.mult)
            nc.vector.tensor_tensor(out=eqc[:], in0=eqc[:],
                                    in1=ct[:, C_UT:C_UT + S], op=OP.mult)
            ignov = tt([S, 1], "ignov")
            nc.vector.tensor_reduce(ignov[:], eqc[:], AX.X, OP.max)
            # weff = weight*(1-ignov) - ignov
            weff = tt([S, 1], "weff")
            nc.vector.tensor_scalar(weff[:], ignov[:], -1.0, 1.0,
                                    OP.mult, OP.add)
            nc.vector.tensor_tensor(out=weff[:], in0=weff[:], in1=weight[:],
                                    op=OP.mult)
            nc.vector.tensor_tensor(out=weff[:], in0=weff[:], in1=ignov[:],
                                    op=OP.subtract)

            # ---------------- indirect gather of 85 channels ---------------
            u3 = tt([S, 3], "u3")
            nc.scalar.activation(u3[:], pf[:, 0:3], AF.Exp, scale=-1.0)
            sig3 = tt([S, 3], "sig3")
            nc.vector.tensor_scalar(sig3[:], u3[:], 1.0, None, OP.add)
            nc.vector.reciprocal(sig3[:], sig3[:])
            pconf = sig3[:, 0:1]
            sxy = sig3[:, 1:3]
            pxy = tt([S, 2], "pxy")
            nc.vector.tensor_tensor(out=pxy[:], in0=sxy, in1=colrow[:],
                                    op=OP.add)
            nc.vector.tensor_scalar_mul(pxy[:], pxy[:], 1.0 / GRID)
            pwh = tt([S, 2], "pwh")
            nc.scalar.activation(pwh[:], pf[:, 3:5], AF.Exp)
            nc.vector.tensor_tensor(out=pwh[:], in0=pwh[:], in1=ancsel[:],
                                    op=OP.mult)
            pwh2 = tt([S, 2], "pwh2")
            nc.vector.tensor_scalar_mul(pwh2[:], pwh[:], 0.5)
            plt = tt([S, 2], "plt")
            nc.vector.tensor_tensor(out=plt[:], in0=pxy[:], in1=pwh2[:],
                                    op=OP.subtract)
            prb = tt([S, 2], "prb")
            nc.vector.tensor_tensor(out=prb[:], in0=pxy[:], in1=pwh2[:],
                                    op=OP.add)
            ilt = tt([S, 2], "ilt")
            nc.vector.tensor_tensor(out=ilt[:], in0=plt[:],
                                    in1=ct[:, C_GB:C_GB + 2],
                                    op=OP.max)
            irb = tt([S, 2], "irb")
            nc.vector.tensor_tensor(out=irb[:], in0=prb[:],
                                    in1=ct[:, C_GB + 2:C_GB + 4],
                                    op=OP.min)
            iwh = tt([S, 2], "iwh")
            nc.vector.tensor_tensor(out=iwh[:], in0=irb[:], in1=ilt[:],
                                    op=OP.subtract)
            nc.vector.tensor_scalar(iwh[:], iwh[:], 0.0, None, OP.max)
            inter2 = tt([S, 1], "inter2")
            nc.vector.tensor_tensor(out=inter2[:], in0=iwh[:, 0:1],
                                    in1=iwh[:, 1:2], op=OP.mult)
            pa = tt([S, 1], "pa")
            nc.vector.tensor_tensor(out=pa[:], in0=pwh[:, 0:1],
                                    in1=pwh[:, 1:2], op=OP.mult)
            den2 = tt([S, 1], "den2")
            nc.vector.tensor_tensor(out=den2[:], in0=areag[:], in1=inter2[:],
                                    op=OP.subtract)
            nc.vector.tensor_scalar(den2[:], den2[:], pa[:, 0:1], EPS,
                                    OP.add, OP.add)
            den2i = tt([S, 1], "den2i")
            nc.vector.reciprocal(den2i[:], den2[:])
            gconf = tt([S, 1], "gconf")
            nc.vector.tensor_tensor(out=gconf[:], in0=inter2[:], in1=den2i[:],
                                    op=OP.mult)
            gpos = tt([S, 1], "gpos")
            nc.vector.tensor_scalar(gpos[:], gconf[:], 0.0, None, OP.is_gt)
            mp = tt([S, 1], "mp")
            nc.vector.tensor_tensor(out=mp[:], in0=lastw[:], in1=gpos[:],
                                    op=OP.mult)
            mpw = tt([S, 1], "mpw")
            nc.vector.tensor_tensor(out=mpw[:], in0=mp[:], in1=weff[:],
                                    op=OP.mult)

            stack = sb.tile([S, 8], F32, name="stack")
            nc.vector.memset(stack[:], 0.0)
            nc.vector.tensor_copy(stack[0:BL * 5, 0:1], densesq[:])

            dconf = tt([S, 1], "dconf")
            nc.vector.tensor_scalar(dconf[:], pconf, gconf[:, 0:1], None,
                                    OP.subtract)
            nc.vector.tensor_tensor(out=dconf[:], in0=dconf[:], in1=dconf[:],
                                    op=OP.mult)
            nc.vector.tensor_tensor(out=stack[:, 1:2], in0=mp[:],
                                    in1=dconf[:], op=OP.mult)
            psq = tt([S, 1], "psq")
            nc.vector.tensor_tensor(out=psq[:], in0=pconf, in1=pconf,
                                    op=OP.mult)
            nc.vector.tensor_tensor(out=stack[:, 2:3], in0=mp[:], in1=psq[:],
                                    op=OP.mult)
            nc.vector.tensor_copy(stack[:, 3:4], mp[:])

            # cls: sum softplus(x_c) - x_label over channels 5..85
            sp80 = tt([S, 80], "sp80")
            spsum = tt([S, 1], "spsum")
            nc.scalar.activation(sp80[:], pf[:, 5:85], AF.Exp)
            nc.scalar.activation(sp80[:], sp80[:], AF.Ln, bias=1.0,
                                 accum_out=spsum[:])   # softplus
            lblm1 = tt([S, 1], "lblm1")
            nc.vector.tensor_scalar(lblm1[:], ct[:, C_LBL:C_LBL + 1], -1.0,
                                    None, OP.add)
            oh80 = tt([S, 80], "oh80")
            nc.vector.tensor_scalar(oh80[:], ct[:, C_IOTA80:C_IOTA80 + 80],
                                    lblm1[:, 0:1], None, OP.is_equal)
            xlab = tt([S, 1], "xlab")
            scr80 = tt([S, 80], "scr80")
            nc.vector.tensor_tensor(out=scr80[:], in0=oh80[:],
                                    in1=pf[:, 5:85], op=OP.mult)
            nc.vector.tensor_reduce(xlab[:], scr80[:], AX.X, OP.add)
            clsn = tt([S, 1], "clsn")
            nc.vector.tensor_tensor(out=clsn[:], in0=spsum[:], in1=xlab[:],
                                    op=OP.subtract)
            nc.vector.tensor_tensor(out=stack[:, 4:5], in0=mp[:], in1=clsn[:],
                                    op=OP.mult)

            # txy bce: softplus(x) - z*x = x + softplus(-x) - z*x; reuse u3
            sptxy = tt([S, 2], "sptxy")
            nc.scalar.activation(sptxy[:], u3[:, 1:3], AF.Ln, bias=1.0)
            nc.vector.tensor_tensor(out=sptxy[:], in0=sptxy[:],
                                    in1=pf[:, 1:3], op=OP.add)
            zx = tt([S, 2], "zx")
            nc.vector.tensor_tensor(out=zx[:], in0=txy[:], in1=pf[:, 1:3],
                                    op=OP.mult)
            nc.vector.tensor_tensor(out=sptxy[:], in0=sptxy[:], in1=zx[:],
                                    op=OP.subtract)
            bcexy = tt([S, 1], "bcexy")
            nc.vector.tensor_reduce(bcexy[:], sptxy[:], AX.X, OP.add)
            nc.vector.tensor_tensor(out=stack[:, 5:6], in0=mpw[:],
                                    in1=bcexy[:], op=OP.mult)

            # twh mse on channels 3:5
            dwh = tt([S, 2], "dwh")
            nc.vector.tensor_tensor(out=dwh[:], in0=pf[:, 3:5], in1=twh[:],
                                    op=OP.subtract)
            nc.vector.tensor_tensor(out=dwh[:], in0=dwh[:], in1=dwh[:],
                                    op=OP.mult)
            msewh = tt([S, 1], "msewh")
            nc.vector.tensor_reduce(msewh[:], dwh[:], AX.X, OP.add)
            nc.vector.tensor_tensor(out=stack[:, 6:7], in0=mpw[:],
                                    in1=msewh[:], op=OP.mult)

            # ---------------- cross-partition reduce + out ----------------
            red = ps.tile([1, 8], F32, name="red")
            nc.tensor.matmul(out=red[:], lhsT=ct[:, C_ONES:C_ONES + 1],
                             rhs=stack[:], start=True, stop=True)
            osb = sb.tile([1, 8], F32, name="osb")
            nc.vector.tensor_copy(osb[:], red[:])
            nc.sync.dma_start(out=out[:, :], in_=osb[:])
    _split_multiwaits(nc, k=1)
    return nc


_NC_CACHE = None
LAST_RESULTS = None


def _get_nc():
    global _NC_CACHE
    if _NC_CACHE is None:
        _NC_CACHE = build_bass()
    return _NC_CACHE


def run(pyolos, gboxes_ltrb, labels, trace=False, **spmd_kwargs):
    global LAST_RESULTS
    nc = _get_nc()
    py = np.ascontiguousarray(
        np.asarray(pyolos, np.float32).reshape(B, 425, HW))
    gbx = np.ascontiguousarray(np.asarray(gboxes_ltrb, np.float32))
    lbl = np.asarray(labels).astype(np.float32)
    in_maps = []
    for c in range(NC):
        sl = slice(c * BL, (c + 1) * BL)
        in_maps.append({
            "pyolos": py[sl],
            "consts": _make_consts(gbx[sl].reshape(S, 4),
                                   lbl[sl].reshape(S)),
        })
    res = run_bass_kernel_spmd(nc, in_maps, list(range(NC)), trace=trace,
                               **spmd_kwargs)
    LAST_RESULTS = res
    outs = np.stack([r["out"][0] for r in res.results]).astype(np.float64)
    t = outs.sum(0)
    dense_sq, pos_mse, pos_psq, npos, cls_num, txy_s, twh_s = t[:7]
    loss = (5.0 * pos_mse / B
            + (dense_sq - pos_psq) / B
            + cls_num / max(npos, 1.0)
            + txy_s / B
            + twh_s / B)
    return np.float32(loss)


def kernel(pyolos, gboxes_ltrb, labels):
    return run(pyolos, gboxes_ltrb, labels)



# revision 8
# speedup vs baseline: 1.2503x; 1.2503x over previous
"""YOLO-v2 loss kernel for Trainium2 (8 NeuronCores, data-parallel over batch).

Decomposition (same partial sums as the validated baseline):
  stack cols = [dense_sigmoid_sq, pos_mse, pos_psq, npos, cls_num, txy, twh]
summed over 128 partitions by one PE matmul, combined on the host.

The GT matching (anchors IoU, argmax, cell assignment, last-writer-wins and
ignore-overwrite masks) depends only on gboxes/labels (4KB of input), so it is
precomputed on the host into per-slot constants + gather offsets, exactly like
the baseline already precomputed its per-slot channel offsets.  Everything
touching pyolos (the 147MB tensor) stays on-device:
  - conf planes (16x5x676 per core) DMA'd + 3-pass sigmoid^2-accumulate on ACT
  - 90 channel values per slot fetched with a 2-stage indirect DMA
  - decode/IoU/loss math on DVE+Pool in parallel strands

Key trick: columns are gathered in the order
  [conf, tx, ty, tx, ty, tw, th, tw, th, lbl_ch, cls0..79]
with tx/ty duplicated so that one Exp(-x) and one Exp(+x) activation yield
both sigmoid(t) and sigmoid(-t) = 1 - sigmoid(t); the prb / -plt box corners
then come out of a single add against host-folded constants, and the IoU
min/max pair collapses to one tensor_tensor min.
"""

import numpy as np

from concourse import bass, mybir
from concourse.bass_utils import run_bass_kernel_spmd
from concourse.tile import TileContext

F32 = mybir.dt.float32
I32 = mybir.dt.int32
AF = mybir.ActivationFunctionType
OP = mybir.AluOpType
AX = mybir.AxisListType

NC = 8                 # cores
B = 128                # batch
BL = B // NC           # images per core (16)
NGT = 8                # GTs per image
S = BL * NGT           # slots per core (128)
GRID = 26
HW = GRID * GRID       # 676
IMG = 425 * HW         # elements per image
EPS = 1e-7
NCOL = 90              # gathered columns per slot
NCONST = 24            # f32 const columns
NMETA = NCOL + NCONST  # i32 meta columns (consts bitcast)
ANC = np.array([[0.05, 0.07], [0.12, 0.15], [0.25, 0.30],
                [0.45, 0.50], [0.80, 0.85]], np.float32)

# const f32 column offsets (within the 24-col block)
C_C4 = 0      # [4]  [cr0, cr1, -1-cr0, -1-cr1]
C_AH4 = 4     # [4]  [aw, ah, aw, ah] * 13
C_GB4 = 8     # [4]  [r, b, -l, -t] * 26
C_AGE = 12    # [1]  gt area*676 + eps*676
C_LW = 13     # [1]  last-writer mask
C_LWE = 14    # [1]  lw * weff
C_ZC = 15     # [2]  1 - txy target
C_TWT = 17    # [2]  twh target
C_ONE = 19    # [1]  1.0


def _split_multiwaits(nc: bass.Bass, k: int = 1) -> None:
    """This walrus build rejects instructions with >~2 sync waits; hoist
    extra waits onto preceding same-engine NoOps."""
    for fn in nc.m.functions:
        for bb in fn.blocks:
            out = []
            for inst in bb.instructions:
                si = inst.sync_info
                waits = list(si.on_wait) if si is not None and si.on_wait else []
                if len(waits) > k:
                    for i, w in enumerate(waits[:-k]):
                        out.append(mybir.InstNoOp(
                            name=f"{inst.name}-wsplit{i}",
                            engine=inst.engine,
                            bass_nofuse=True,
                            sync_info=mybir.SyncInfo(on_wait=[w],
                                                     on_update=[]),
                        ))
                    inst.sync_info = mybir.SyncInfo(
                        on_wait=waits[-k:], on_update=list(si.on_update))
                out.append(inst)
            bb.instructions = out


def _host_match(gbx: np.ndarray, lbl: np.ndarray):
    """Vectorized fmatch4yolov2 mirror (f32, matches the jax reference).
    gbx [B,8,4] ltrb, lbl [B,8] 1-based.  Returns per-slot meta arrays."""
    gbx = gbx.astype(np.float32)
    cxy = (gbx[..., :2] + gbx[..., 2:]) * np.float32(0.5)
    wh = gbx[..., 2:] - gbx[..., :2]
    inter = np.minimum(wh[..., None, :], ANC[None, None]).prod(-1)
    areag = wh.prod(-1)
    iou2 = inter / (areag[..., None] + (ANC[:, 0] * ANC[:, 1])[None, None]
                    - inter + np.float32(EPS))
    mign = iou2 > 0.5                                   # [B,8,5]
    idxm = iou2.argmax(-1)                              # [B,8]
    colrow = (cxy * np.float32(GRID)).astype(np.int32)  # trunc == floor here
    crf = colrow.astype(np.float32)
    txy = cxy * np.float32(GRID) - crf
    twh = np.log(wh / ANC[idxm])
    weight = np.float32(2.0) - areag
    cell = colrow[..., 1] * GRID + colrow[..., 0]       # [B,8] int
    key = cell * 5 + idxm

    # upper-triangular (j > i) collision masks
    jgt = np.triu(np.ones((NGT, NGT), bool), 1)[None]   # [1,i,j]
    same_key = key[:, :, None] == key[:, None, :]       # [b,i,j]
    lastw = ~np.logical_and(same_key, jgt).any(-1)      # [B,8]
    same_cell = cell[:, :, None] == cell[:, None, :]
    # mji[b,i,j] = mign[b, j, idxm[b, i]]
    mji = np.take_along_axis(
        mign.transpose(0, 2, 1),                        # [b, a, j]
        idxm[:, :, None], axis=1)                       # [b, i, j]
    ignov = np.logical_and(np.logical_and(same_cell, jgt), mji).any(-1)
    weff = np.where(ignov, np.float32(-1.0), weight)
    return dict(idxm=idxm, crf=crf, txy=txy, twh=twh, cell=cell,
                lastw=lastw.astype(np.float32), weff=weff.astype(np.float32),
                areag=areag, gbx=gbx)


def _make_meta(m: dict, sl: slice) -> np.ndarray:
    """Pack per-slot gather offsets + f32 consts for one core -> i32 [S,NMETA]."""
    idxm = m["idxm"][sl].reshape(S)
    cell = m["cell"][sl].reshape(S)
    crf = m["crf"][sl].reshape(S, 2)
    txy = m["txy"][sl].reshape(S, 2)
    twh = m["twh"][sl].reshape(S, 2)
    lastw = m["lastw"][sl].reshape(S)
    weff = m["weff"][sl].reshape(S)
    areag = m["areag"][sl].reshape(S)
    gbx = m["gbx"][sl].reshape(S, 4)
    lblch = m["lblch"][sl].reshape(S)

    img = np.arange(S) // NGT
    base = img * IMG + idxm * HW + cell                 # [S]
    # channel planes: conf=0, cls k -> 1+k, tx..th -> 81..84
    ch = np.concatenate([
        np.array([0, 81, 82, 81, 82, 83, 84, 83, 84]),
        np.zeros(1, np.int64),                          # placeholder for lbl
        np.arange(1, 81),
    ])
    offs = base[:, None] + ch[None, :] * (5 * HW)
    offs[:, 9] = base + lblch * (5 * HW)
    meta = np.zeros((S, NMETA), np.int32)
    meta[:, :NCOL] = offs.astype(np.int32)

    ct = np.zeros((S, NCONST), np.float32)
    anc = ANC[idxm]                                     # [S,2]
    ct[:, C_C4 + 0:C_C4 + 2] = crf
    ct[:, C_C4 + 2:C_C4 + 4] = -1.0 - crf
    ct[:, C_AH4 + 0:C_AH4 + 2] = anc * (GRID / 2.0)
    ct[:, C_AH4 + 2:C_AH4 + 4] = anc * (GRID / 2.0)
    ct[:, C_GB4 + 0] = gbx[:, 2] * GRID
    ct[:, C_GB4 + 1] = gbx[:, 3] * GRID
    ct[:, C_GB4 + 2] = -gbx[:, 0] * GRID
    ct[:, C_GB4 + 3] = -gbx[:, 1] * GRID
    ct[:, C_AGE] = areag * (HW * 1.0) + EPS * HW
    ct[:, C_LW] = lastw
    ct[:, C_LWE] = lastw * weff
    ct[:, C_ZC:C_ZC + 2] = 1.0 - txy
    ct[:, C_TWT:C_TWT + 2] = twh
    ct[:, C_ONE] = 1.0
    meta[:, NCOL:] = ct.view(np.int32)
    return meta


def build_bass() -> bass.Bass:
    nc = bass.Bass()
    py = nc.declare_dram_parameter("pyolos", [BL, 425, HW], F32, isOutput=False)
    mtd = nc.declare_dram_parameter("meta", [S, NMETA], I32, isOutput=False)
    out = nc.declare_dram_parameter("out", [1, 8], F32, isOutput=True)
    py_flat = py[:, :, :].rearrange("a b c -> (a b c)")

    with TileContext(nc) as tc:
        with (
            tc.tile_pool(name="sb", bufs=1) as sb,
            tc.tile_pool(name="ps", bufs=1, space="PSUM") as ps,
        ):
            mt = sb.tile([S, NMETA], I32, name="mt")
            conf = sb.tile([BL * 5, HW], F32, name="conf")
            sq = sb.tile([BL * 5, HW], F32, name="sq")
            pf = sb.tile([S, NCOL], F32, name="pf")
            u9 = sb.tile([S, 9], F32, name="u9")
            sp80 = sb.tile([S, 80], F32, name="sp80")
            stack = sb.tile([S, 8], F32, name="stack")

            def ctf(c0, c1):
                return mt[:, NCOL + c0:NCOL + c1].bitcast(F32)

            def tt(shape, tag):
                return sb.tile(shape, F32, name=tag)

            # ---- DMAs: meta on SP queue; conf on Act HWDGE queue ----
            nc.sync.dma_start(out=mt[:], in_=mtd[:, :])
            nc.scalar.dma_start(out=conf[:], in_=py[:, 0:5, :])

            # ---- indirect gathers on gpsimd (stage A: 10 hot cols) ----
            in1 = py_flat.rearrange("(a b) -> a b", b=1)
            nc.gpsimd.indirect_dma_start(
                out=pf[:, 0:10], out_offset=None, in_=in1,
                in_offset=bass.IndirectOffsetOnAxis(ap=mt[:, 0:10], axis=0))
            nc.gpsimd.indirect_dma_start(
                out=pf[:, 10:NCOL], out_offset=None, in_=in1,
                in_offset=bass.IndirectOffsetOnAxis(ap=mt[:, 10:NCOL], axis=0))

            # ---- DVE early (independent of gathers): zero the stack ----
            nc.vector.memset(stack[:], 0.0)

            # ---- Activation queue (single exp/ln table set) ----
            # dense conf background: sigmoid(x)^2 = exp(-2*ln(1+exp(-x)))
            nc.scalar.activation(sq[:], conf[:], AF.Exp, scale=-1.0)
            nc.scalar.activation(u9[:, 0:3], pf[:, 0:3], AF.Exp, scale=-1.0)
            nc.scalar.activation(u9[:, 3:9], pf[:, 3:9], AF.Exp)
            spl = tt([S, 2], "spl")
            nc.scalar.activation(spl[:], u9[:, 1:3], AF.Ln, bias=1.0)
            nc.scalar.activation(sq[:], sq[:], AF.Ln, bias=1.0)
            spsum = tt([S, 1], "spsum")
            nc.scalar.activation(sp80[:], pf[:, 10:NCOL], AF.Exp)
            nc.scalar.activation(sp80[:], sp80[:], AF.Ln, bias=1.0,
                                 accum_out=spsum[:])
            nc.scalar.activation(sq[:], sq[:], AF.Exp, scale=-2.0,
                                 accum_out=stack[0:BL * 5, 0:1])

            # ---- DVE: independent per-slot bits ----
            t2 = tt([S, 2], "t2")
            nc.vector.tensor_tensor(out=t2[:], in0=pf[:, 1:3],
                                    in1=ctf(C_ZC, C_ZC + 2), op=OP.mult)
            dwh = tt([S, 2], "dwh")
            nc.vector.tensor_tensor(out=dwh[:], in0=pf[:, 5:7],
                                    in1=ctf(C_TWT, C_TWT + 2), op=OP.subtract)
            dsq = tt([S, 2], "dsq")
            msewh = tt([S, 1], "msewh")
            nc.vector.tensor_tensor(out=dsq[:], in0=dwh[:], in1=dwh[:],
                                    op=OP.mult)
            nc.vector.tensor_reduce(msewh[:], dsq[:], AX.X, OP.add)
            v5 = tt([S, 5], "v5")
            nc.vector.tensor_scalar(v5[:], u9[:, 0:5], 1.0, None, OP.add)
            sig5 = tt([S, 5], "sig5")
            nc.vector.reciprocal(sig5[:], v5[:])

            # ---- Pool strand: pwh/area pieces feeding the DVE chain ----
            w4 = tt([S, 4], "w4")
            nc.gpsimd.tensor_tensor(out=w4[:], in0=u9[:, 5:9],
                                    in1=ctf(C_AH4, C_AH4 + 4), op=OP.mult)
            w4c = tt([S, 4], "w4c")
            nc.gpsimd.tensor_tensor(out=w4c[:], in0=w4[:],
                                    in1=ctf(C_C4, C_C4 + 4), op=OP.add)
            pa = tt([S, 1], "pa")
            nc.gpsimd.tensor_tensor(out=pa[:], in0=w4[:, 0:1], in1=w4[:, 1:2],
                                    op=OP.mult)
            pa_ag = tt([S, 1], "pa_ag")
            nc.gpsimd.tensor_scalar(pa_ag[:], pa[:], 4.0,
                                    ctf(C_AGE, C_AGE + 1), OP.mult, OP.add)
            # ---- DVE decode/IoU chain ----
            q4 = tt([S, 4], "q4")
            nc.vector.tensor_tensor(out=q4[:], in0=sig5[:, 1:5], in1=w4c[:],
                                    op=OP.add)
            q4m = tt([S, 4], "q4m")
            nc.vector.tensor_tensor(out=q4m[:], in0=q4[:],
                                    in1=ctf(C_GB4, C_GB4 + 4), op=OP.min)
            s2 = tt([S, 2], "s2")
            nc.vector.tensor_tensor(out=s2[:], in0=q4m[:, 0:2],
                                    in1=q4m[:, 2:4], op=OP.add)
            iwh = tt([S, 2], "iwh")
            nc.vector.tensor_scalar(iwh[:], s2[:], 0.0, None, OP.max)
            inter = tt([S, 1], "inter")
            nc.vector.tensor_tensor(out=inter[:], in0=iwh[:, 0:1],
                                    in1=iwh[:, 1:2], op=OP.mult)
            den = tt([S, 1], "den")
            nc.vector.scalar_tensor_tensor(out=den[:], in0=inter[:],
                                           scalar=-1.0, in1=pa_ag[:],
                                           op0=OP.mult, op1=OP.add)
            deni = tt([S, 1], "deni")
            nc.vector.reciprocal(deni[:], den[:])
            gconf = tt([S, 1], "gconf")
            nc.vector.tensor_tensor(out=gconf[:], in0=inter[:], in1=deni[:],
                                    op=OP.mult)
            # mp = (gconf > 0) * lastw, written straight into the npos col
            nc.vector.tensor_scalar(stack[:, 3:4], gconf[:], 0.0,
                                    ctf(C_LW, C_LW + 1), OP.is_gt, OP.mult)
            dconf = tt([S, 1], "dconf")
            nc.vector.tensor_tensor(out=dconf[:], in0=sig5[:, 0:1],
                                    in1=gconf[:], op=OP.subtract)
            nc.vector.scalar_tensor_tensor(out=stack[:, 1:2], in0=dconf[:],
                                           scalar=stack[:, 3:4], in1=dconf[:],
                                           op0=OP.mult, op1=OP.mult)
            sptxy = tt([S, 2], "sptxy")
            bcexy = tt([S, 1], "bcexy")
            nc.vector.tensor_tensor(out=sptxy[:], in0=spl[:], in1=t2[:],
                                    op=OP.add)
            nc.vector.tensor_reduce(bcexy[:], sptxy[:], AX.X, OP.add)

            # ---- Pool tail: remaining stack columns ----
            clsn = tt([S, 1], "clsn")
            nc.gpsimd.tensor_tensor(out=clsn[:], in0=spsum[:],
                                    in1=pf[:, 9:10], op=OP.subtract)
            mpw = tt([S, 1], "mpw")
            nc.gpsimd.tensor_scalar(mpw[:], gconf[:], 0.0,
                                    ctf(C_LWE, C_LWE + 1), OP.is_gt, OP.mult)
            nc.vector.scalar_tensor_tensor(out=stack[:, 2:3], in0=sig5[:, 0:1],
                                           scalar=stack[:, 3:4],
                                           in1=sig5[:, 0:1],
                                           op0=OP.mult, op1=OP.mult)
            nc.gpsimd.tensor_scalar(stack[:, 4:5], clsn[:], stack[:, 3:4],
                                    None, OP.mult)
            nc.gpsimd.tensor_scalar(stack[:, 5:6], bcexy[:], mpw[:], None,
                                    OP.mult)
            # twh col on DVE (balances the two tails)
            nc.vector.tensor_scalar(stack[:, 6:7], msewh[:], mpw[:], None,
                                    OP.mult)

            # ---- final reduce over partitions + output ----
            red = ps.tile([1, 8], F32, name="red")
            nc.tensor.matmul(out=red[:], lhsT=ctf(C_ONE, C_ONE + 1),
                             rhs=stack[:], start=True, stop=True)
            osb = sb.tile([1, 8], F32, name="osb")
            nc.vector.tensor_copy(osb[:], red[:])
            nc.gpsimd.dma_start(out=out[:, :], in_=osb[:])
    _split_multiwaits(nc, k=1)
    return nc


_NC_CACHE = None
LAST_RESULTS = None


def _get_nc():
    global _NC_CACHE
    if _NC_CACHE is None:
        _NC_CACHE = build_bass()
    return _NC_CACHE


def run(pyolos, gboxes_ltrb, labels, trace=False, **spmd_kwargs):
    global LAST_RESULTS
    nc = _get_nc()
    py = np.ascontiguousarray(
        np.asarray(pyolos, np.float32).reshape(B, 425, HW))
    gbx = np.asarray(gboxes_ltrb, np.float32).reshape(B, NGT, 4)
    lbl = np.asarray(labels).reshape(B, NGT).astype(np.int64)
    m = _host_match(gbx, lbl)
    m["lblch"] = lbl  # class channel plane index is exactly the 1-based label
    in_maps = []
    for c in range(NC):
        sl = slice(c * BL, (c + 1) * BL)
        in_maps.append({
            "pyolos": py[sl],
            "meta": _make_meta(m, sl),
        })
    res = run_bass_kernel_spmd(nc, in_maps, list(range(NC)), trace=trace,
                               **spmd_kwargs)
    LAST_RESULTS = res
    outs = np.stack([r["out"][0] for r in res.results]).astype(np.float64)
    t = outs.sum(0)
    dense_sq, pos_mse, pos_psq, npos, cls_num, txy_s, twh_s = t[:7]
    loss = (5.0 * pos_mse / B
            + (dense_sq - pos_psq) / B
            + cls_num / max(npos, 1.0)
            + txy_s / B
            + twh_s / B)
    return np.float32(loss)


def kernel(pyolos, gboxes_ltrb, labels):
    return run(pyolos, gboxes_ltrb, labels)
